# revision 3
# baseline (speedup 1.0000x reference)
"""Multi-head attention (B=4, S=2048, D=512, H=8) on 8 Trainium2 NeuronCores.

Sharding: core c handles batch b = c//2 and query-half h = c%2 (1024 queries).
Each core computes q = (x_q @ Wq.T + bq)/sqrt(hd) for its queries, k/v
projections for its batch's full 2048 keys, full softmax attention for all 8
heads, and the output projection for its query rows.  Output rows across
cores are disjoint, so there are no collectives.

On-chip layout is feature-major ("transposed activations"): scores are built
directly as S^T[k, q] so the attn @ V contraction needs no transposes, and
exp(S^T) row-sums come free via a ones-column appended to V.
"""

import numpy as np
import ml_dtypes

B = 4
S = 2048
D = 512
H = 8
HD = 64
SQ = 1024  # queries per core
N_CORES = 8
F32 = None  # set lazily (mybir dtypes) in _build
BF16 = None

_cache = {}


def _build():
    """Build (once) the SPMD Bass program shared by all 8 cores."""
    import concourse.bacc as bacc
    import concourse.mybir as mybir
    import concourse.tile as tile

    f32 = mybir.dt.float32
    bf16 = mybir.dt.bfloat16
    AF = mybir.ActivationFunctionType
    OP = mybir.AluOpType

    nc = bacc.Bacc("TRN2", target_bir_lowering=False, debug=False)

    # Per-core inputs (pre-transposed / pre-cast on host).
    xqT = nc.dram_tensor("xqT", [D, SQ], bf16, kind="ExternalInput").ap()
    keyT = nc.dram_tensor("keyT", [D, S], bf16, kind="ExternalInput").ap()
    valT = nc.dram_tensor("valT", [D, S], bf16, kind="ExternalInput").ap()
    wqT = nc.dram_tensor("wqT", [D, D], bf16, kind="ExternalInput").ap()
    wkT = nc.dram_tensor("wkT", [D, D], bf16, kind="ExternalInput").ap()
    wvT = nc.dram_tensor("wvT", [D, D], bf16, kind="ExternalInput").ap()
    woT = nc.dram_tensor("woT", [D, D], bf16, kind="ExternalInput").ap()
    bqr = nc.dram_tensor("bqr", [128, 4], f32, kind="ExternalInput").ap()
    bkr = nc.dram_tensor("bkr", [128, 4], f32, kind="ExternalInput").ap()
    bop = nc.dram_tensor("bop", [1, D], bf16, kind="ExternalInput").ap()
    y = nc.dram_tensor("y", [SQ, D], f32, kind="ExternalOutput").ap()

    with tile.TileContext(nc) as tc:
        import contextlib

        with contextlib.ExitStack() as ctx:
            const = ctx.enter_context(tc.tile_pool(name="const", bufs=1))
            io = ctx.enter_context(tc.tile_pool(name="io", bufs=1))
            acts = ctx.enter_context(tc.tile_pool(name="acts", bufs=1))
            expp = ctx.enter_context(tc.tile_pool(name="expp", bufs=8))
            rpool = ctx.enter_context(tc.tile_pool(name="rpool", bufs=2))
            dramp = ctx.enter_context(
                tc.tile_pool(name="dramp", bufs=2, space="DRAM")
            )
            psA = ctx.enter_context(tc.tile_pool(name="psA", bufs=2, space="PSUM"))
            psB = ctx.enter_context(tc.tile_pool(name="psB", bufs=4, space="PSUM"))

            # ---- constants / weights -------------------------------------
            wq_sb = const.tile([128, 4, D], bf16)
            wk_sb = const.tile([128, 4, D], bf16)
            wv_sb = const.tile([128, 4, D], bf16)
            wo_sb = const.tile([128, 4, D], bf16)
            for w_sb, w_dr in ((wq_sb, wqT), (wk_sb, wkT), (wv_sb, wvT), (wo_sb, woT)):
                nc.sync.dma_start(w_sb[:], w_dr.rearrange("(c p) e -> p c e", p=128))
            bq_sb = const.tile([128, 4], f32)
            bk_sb = const.tile([128, 4], f32)
            nc.sync.dma_start(bq_sb[:], bqr[:])
            nc.sync.dma_start(bk_sb[:], bkr[:])
            bop_sb = const.tile([1, D], bf16)
            nc.sync.dma_start(bop_sb[:], bop[:])
            ones_row = const.tile([1, 128], bf16)
            nc.vector.memset(ones_row[:], 1.0)

            # ---- inputs ---------------------------------------------------
            xq_sb = io.tile([128, 4, SQ], bf16)
            key_sb = io.tile([128, 4, S], bf16)
            val_sb = io.tile([128, 4, S], bf16)
            nc.sync.dma_start(xq_sb[:], xqT.rearrange("(c p) s -> p c s", p=128))
            nc.sync.dma_start(key_sb[:], keyT.rearrange("(c p) s -> p c s", p=128))
            nc.sync.dma_start(val_sb[:], valT.rearrange("(c p) s -> p c s", p=128))

            # ---- projections ---------------------------------------------
            qT_sb = acts.tile([128, 4, SQ], bf16)  # q^T / 8, feature-major
            kT_sb = acts.tile([128, 4, S], bf16)  # k^T, feature-major
            # v natural [s, e] per k-tile, 65th column = 1.0 (row-sum trick)
            v_sb = acts.tile([128, 16, H, HD + 1], bf16)
            nc.vector.memset(v_sb[:, :, :, HD : HD + 1], 1.0)

            # q^T[e, s] = sum_d WqT[d, e] x^T[d, s]
            for et in range(4):
                ps = psA.tile([128, SQ], f32, tag="psA")
                for dc in range(4):
                    for qn in range(2):
                        nc.tensor.matmul(
                            ps[:, qn * 512 : (qn + 1) * 512],
                            lhsT=wq_sb[:, dc, et * 128 : (et + 1) * 128],
                            rhs=xq_sb[:, dc, qn * 512 : (qn + 1) * 512],
                            start=(dc == 0),
                            stop=(dc == 3),
                        )
                nc.vector.tensor_scalar(
                    qT_sb[:, et, :], ps[:], bq_sb[:, et : et + 1], 0.125,
                    OP.add, OP.mult,
                )

            # k^T[e, s] likewise (no scale)
            for et in range(4):
                for kn in range(2):
                    ps = psA.tile([128, SQ], f32, tag="psA")
                    for dc in range(4):
                        for qn in range(2):
                            o = kn * 1024 + qn * 512
                            nc.tensor.matmul(
                                ps[:, qn * 512 : (qn + 1) * 512],
                                lhsT=wk_sb[:, dc, et * 128 : (et + 1) * 128],
                                rhs=key_sb[:, dc, o : o + 512],
                                start=(dc == 0),
                                stop=(dc == 3),
                            )
                    nc.vector.tensor_scalar(
                        kT_sb[:, et, kn * 1024 : (kn + 1) * 1024], ps[:],
                        bk_sb[:, et : et + 1], None, OP.add,
                    )

            # v[s, e] = sum_d v^T[d, s] WvT[d, e]   (bias folded into bo')
            for st in range(16):
                psv = psB.tile([128, 512], f32, tag="psB")
                for dc in range(4):
                    nc.tensor.matmul(
                        psv[:],
                        lhsT=val_sb[:, dc, st * 128 : (st + 1) * 128],
                        rhs=wv_sb[:, dc, :],
                        start=(dc == 0),
                        stop=(dc == 3),
                    )
                nc.vector.tensor_copy(
                    v_sb[:, st, :, 0:HD],
                    psv[:].rearrange("p (h d) -> p h d", h=H),
                )

            # ---- attention (head pairs share one 128-row tile) ------------
            outT = []  # 4 pair tiles [128, SQ] = attn-out^T, normalized
            for hp in range(4):
                pair_out = acts.tile([128, SQ], bf16, tag=f"outT{hp}")
                outT.append(pair_out)
                av = [
                    [
                        psB.tile(
                            [HD + 1, 512], f32, tag="psB",
                            name=f"av{hp}_{hh}_{qc}",
                        )
                        for qc in range(2)
                    ]
                    for hh in range(2)
                ]
                exp_tiles = [[None] * 16, [None] * 16]
                for kt in range(16):
                    st_ps = [None, None]
                    for hh in range(2):
                        lo = 64 * hh
                        st_ps[hh] = psA.tile(
                            [128, SQ], f32, tag="psA", name=f"st{hp}_{kt}_{hh}"
                        )
                        for qn in range(2):
                            nc.tensor.matmul(
                                st_ps[hh][:, qn * 512 : (qn + 1) * 512],
                                lhsT=kT_sb[lo : lo + 64, hp, kt * 128 : (kt + 1) * 128],
                                rhs=qT_sb[lo : lo + 64, hp, qn * 512 : (qn + 1) * 512],
                                start=True,
                                stop=True,
                            )
                    for hh in range(2):
                        e = expp.tile([128, SQ], bf16, tag="exp")
                        exp_tiles[hh][kt] = e
                        nc.scalar.activation(e[:], st_ps[hh][:], AF.Exp)
                    for hh in range(2):
                        h = 2 * hp + hh
                        for qc in range(2):
                            nc.tensor.matmul(
                                av[hh][qc][:],
                                lhsT=v_sb[:, kt, h, :],
                                rhs=exp_tiles[hh][kt][:, qc * 512 : (qc + 1) * 512],
                                start=(kt == 0),
                                stop=(kt == 15),
                            )
                # normalize: out^T[dh, q] = av[dh, q] / av[64, q]
                for hh in range(2):
                    rrow = rpool.tile([1, SQ], f32, tag="rrow")
                    for qc in range(2):
                        nc.vector.reciprocal(
                            rrow[:, qc * 512 : (qc + 1) * 512],
                            av[hh][qc][HD : HD + 1, :],
                        )
                    scr = dramp.tile([1, SQ], f32, tag="scr")
                    nc.sync.dma_start(scr[:], rrow[:])
                    rb = rpool.tile([64, SQ], f32, tag="rb")
                    nc.sync.dma_start(rb[:], scr[:].to_broadcast((64, SQ)))
                    for qc in range(2):
                        nc.vector.tensor_tensor(
                            pair_out[64 * hh : 64 * hh + 64, qc * 512 : (qc + 1) * 512],
                            av[hh][qc][0:HD, :],
                            rb[:, qc * 512 : (qc + 1) * 512],
                            OP.mult,
                        )

            # ---- output projection ---------------------------------------
            # y[q, o] = sum_e outT[e, q] WoT[e, o] + bo'
            for stq in range(8):
                psy = psA.tile([128, 512], f32, tag="psA")
                for c in range(4):
                    nc.tensor.matmul(
                        psy[:],
                        lhsT=outT[c][:, stq * 128 : (stq + 1) * 128],
                        rhs=wo_sb[:, c, :],
                        start=(c == 0),
                        stop=False,
                    )
                nc.tensor.matmul(
                    psy[:], lhsT=ones_row[:], rhs=bop_sb[:], start=False, stop=True,
                )
                ysb = rpool.tile([128, 512], f32, tag="ysb", name=f"ysb{stq}")
                nc.vector.tensor_copy(ysb[:], psy[:])
                nc.sync.dma_start(y[stq * 128 : (stq + 1) * 128, :], ysb[:])

    nc.compile()
    return nc


def _get_nc():
    if "nc" not in _cache:
        _cache["nc"] = _build()
    return _cache["nc"]


def _host_prep(query, key, value, Wq, bq, Wk, bk, Wv, bv, Wo, bo):
    """Shard + transpose + cast inputs for the 8 cores."""
    bf = ml_dtypes.bfloat16
    wqT = np.ascontiguousarray(Wq.T).astype(bf)
    wkT = np.ascontiguousarray(Wk.T).astype(bf)
    wvT = np.ascontiguousarray(Wv.T).astype(bf)
    woT = np.ascontiguousarray(Wo.T).astype(bf)
    bqr = np.ascontiguousarray(bq.reshape(4, 128).T).astype(np.float32)
    bkr = np.ascontiguousarray(bk.reshape(4, 128).T).astype(np.float32)
    bop = (bo + Wo @ bv).astype(np.float32).reshape(1, D).astype(bf)

    in_maps = []
    for c in range(N_CORES):
        b, half = divmod(c, 2)
        xqT = np.ascontiguousarray(
            query[b, half * SQ : (half + 1) * SQ, :].T
        ).astype(bf)
        keyT = np.ascontiguousarray(key[b].T).astype(bf)
        valT = np.ascontiguousarray(value[b].T).astype(bf)
        in_maps.append(
            {
                "xqT": xqT, "keyT": keyT, "valT": valT,
                "wqT": wqT, "wkT": wkT, "wvT": wvT, "woT": woT,
                "bqr": bqr, "bkr": bkr, "bop": bop,
            }
        )
    return in_maps


def _assemble(results):
    out = np.empty((B, S, D), np.float32)
    for c in range(N_CORES):
        b, half = divmod(c, 2)
        out[b, half * SQ : (half + 1) * SQ, :] = results[c]["y"]
    return out


def _run(in_maps, **spmd_kwargs):
    from concourse.bass_utils import run_bass_kernel_spmd

    nc = _get_nc()
    return run_bass_kernel_spmd(nc, in_maps, list(range(N_CORES)), **spmd_kwargs)


def _reference_fallback(query, key, value, mask, Wq, bq, Wk, bk, Wv, bv, Wo, bo):
    """Exact numpy path, used only if the mask is not all-ones."""
    q = (query @ Wq.T + bq).reshape(B, S, H, HD).transpose(0, 2, 1, 3)
    k = (key @ Wk.T + bk).reshape(B, S, H, HD).transpose(0, 2, 1, 3)
    v = (value @ Wv.T + bv).reshape(B, S, H, HD).transpose(0, 2, 1, 3)
    scores = np.einsum("bhqd,bhkd->bhqk", q, k) / np.sqrt(HD).astype(np.float32)
    scores = np.where(mask[:, None, :, :] == 0, -np.inf, scores)
    scores = scores - scores.max(axis=-1, keepdims=True)
    e = np.exp(scores)
    attn = e / e.sum(axis=-1, keepdims=True)
    x = np.einsum("bhqk,bhkd->bhqd", attn, v)
    x = x.transpose(0, 2, 1, 3).reshape(B, S, D)
    return (x @ Wo.T + bo).astype(np.float32)


def kernel(query, key, value, mask, Wq, bq, Wk, bk, Wv, bv, Wo, bo):
    query = np.asarray(query, np.float32)
    key = np.asarray(key, np.float32)
    value = np.asarray(value, np.float32)
    mask_np = np.asarray(mask)
    args = [
        np.asarray(a, np.float32)
        for a in (Wq, bq, Wk, bk, Wv, bv, Wo, bo)
    ]
    if not np.all(mask_np != 0):
        return _reference_fallback(query, key, value, mask_np, *args)
    in_maps = _host_prep(query, key, value, *args)
    res = _run(in_maps, trace=False)
    return _assemble(res.results)


# revision 5
# speedup vs baseline: 1.0599x; 1.0599x over previous
"""Multi-head attention (B=4, S=2048, D=512, H=8) on 8 Trainium2 NeuronCores.

Sharding: core c handles batch b = c//2 and query-half h = c%2 (1024 queries).
Each core computes q = (x_q @ Wq.T + bq)/sqrt(hd) for its queries, k/v
projections for its batch's full 2048 keys, full softmax attention for all 8
heads, and the output projection for its query rows.  Output rows across
cores are disjoint, so there are no collectives.

On-chip layout is feature-major ("transposed activations"): scores are built
directly as S^T[k, q] so the attn @ V contraction needs no transposes, and
exp(S^T) row-sums come free via a ones-column appended to V.
"""

import numpy as np
import ml_dtypes

B = 4
S = 2048
D = 512
H = 8
HD = 64
SQ = 1024  # queries per core
N_CORES = 8
F32 = None  # set lazily (mybir dtypes) in _build
BF16 = None

_cache = {}


def _build():
    """Build (once) the SPMD Bass program shared by all 8 cores."""
    import concourse.bacc as bacc
    import concourse.mybir as mybir
    import concourse.tile as tile

    f32 = mybir.dt.float32
    bf16 = mybir.dt.bfloat16
    AF = mybir.ActivationFunctionType
    OP = mybir.AluOpType

    nc = bacc.Bacc("TRN2", target_bir_lowering=False, debug=False)

    # Per-core inputs (pre-transposed / pre-cast on host).
    xqT = nc.dram_tensor("xqT", [D, SQ], bf16, kind="ExternalInput").ap()
    keyT = nc.dram_tensor("keyT", [D, S], bf16, kind="ExternalInput").ap()
    valT = nc.dram_tensor("valT", [D, S], bf16, kind="ExternalInput").ap()
    wqT = nc.dram_tensor("wqT", [D, D], bf16, kind="ExternalInput").ap()
    wkT = nc.dram_tensor("wkT", [D, D], bf16, kind="ExternalInput").ap()
    wvT = nc.dram_tensor("wvT", [D, D], bf16, kind="ExternalInput").ap()
    woT = nc.dram_tensor("woT", [D, D], bf16, kind="ExternalInput").ap()
    bqr = nc.dram_tensor("bqr", [128, 4], f32, kind="ExternalInput").ap()
    bkr = nc.dram_tensor("bkr", [128, 4], f32, kind="ExternalInput").ap()
    bop = nc.dram_tensor("bop", [1, D], bf16, kind="ExternalInput").ap()
    y = nc.dram_tensor("y", [SQ, D], f32, kind="ExternalOutput").ap()

    with tile.TileContext(nc) as tc:
        import contextlib

        with contextlib.ExitStack() as ctx:
            const = ctx.enter_context(tc.tile_pool(name="const", bufs=1))
            io = ctx.enter_context(tc.tile_pool(name="io", bufs=1))
            acts = ctx.enter_context(tc.tile_pool(name="acts", bufs=1))
            expp = ctx.enter_context(tc.tile_pool(name="expp", bufs=8))
            rpool = ctx.enter_context(tc.tile_pool(name="rpool", bufs=2))
            dramp = ctx.enter_context(
                tc.tile_pool(name="dramp", bufs=2, space="DRAM")
            )
            psA = ctx.enter_context(tc.tile_pool(name="psA", bufs=2, space="PSUM"))
            psB = ctx.enter_context(tc.tile_pool(name="psB", bufs=4, space="PSUM"))

            # ---- constants / weights -------------------------------------
            wq_sb = const.tile([128, 4, D], bf16)
            wk_sb = const.tile([128, 4, D], bf16)
            wv_sb = const.tile([128, 4, D], bf16)
            wo_sb = const.tile([128, 4, D], bf16)
            for w_sb, w_dr in ((wq_sb, wqT), (wk_sb, wkT), (wv_sb, wvT), (wo_sb, woT)):
                nc.sync.dma_start(w_sb[:], w_dr.rearrange("(c p) e -> p c e", p=128))
            bq_sb = const.tile([128, 4], f32)
            bk_sb = const.tile([128, 4], f32)
            nc.sync.dma_start(bq_sb[:], bqr[:])
            nc.sync.dma_start(bk_sb[:], bkr[:])
            bop_sb = const.tile([1, D], bf16)
            nc.sync.dma_start(bop_sb[:], bop[:])
            ones_row = const.tile([1, 128], bf16)
            nc.vector.memset(ones_row[:], 1.0)

            # ---- inputs ---------------------------------------------------
            xq_sb = io.tile([128, 4, SQ], bf16)
            key_sb = io.tile([128, 4, S], bf16)
            val_sb = io.tile([128, 4, S], bf16)
            nc.sync.dma_start(xq_sb[:], xqT.rearrange("(c p) s -> p c s", p=128))
            nc.sync.dma_start(key_sb[:], keyT.rearrange("(c p) s -> p c s", p=128))
            nc.sync.dma_start(val_sb[:], valT.rearrange("(c p) s -> p c s", p=128))

            # ---- projections ---------------------------------------------
            qT_sb = acts.tile([128, 4, SQ], bf16)  # q^T / 8, feature-major
            kT_sb = acts.tile([128, 4, S], bf16)  # k^T, feature-major
            # v natural [s, e] per k-tile, 65th column = 1.0 (row-sum trick)
            v_sb = acts.tile([128, 16, H, HD + 1], bf16)
            nc.vector.memset(v_sb[:, :, :, HD : HD + 1], 1.0)

            # q^T[e, s] = sum_d WqT[d, e] x^T[d, s]
            for et in range(4):
                ps = psA.tile([128, SQ], f32, tag="psA")
                for dc in range(4):
                    for qn in range(2):
                        nc.tensor.matmul(
                            ps[:, qn * 512 : (qn + 1) * 512],
                            lhsT=wq_sb[:, dc, et * 128 : (et + 1) * 128],
                            rhs=xq_sb[:, dc, qn * 512 : (qn + 1) * 512],
                            start=(dc == 0),
                            stop=(dc == 3),
                        )
                nc.vector.tensor_scalar(
                    qT_sb[:, et, :], ps[:], bq_sb[:, et : et + 1], 0.125,
                    OP.add, OP.mult,
                )

            # k^T[e, s] likewise (no scale)
            for et in range(4):
                for kn in range(2):
                    ps = psA.tile([128, SQ], f32, tag="psA")
                    for dc in range(4):
                        for qn in range(2):
                            o = kn * 1024 + qn * 512
                            nc.tensor.matmul(
                                ps[:, qn * 512 : (qn + 1) * 512],
                                lhsT=wk_sb[:, dc, et * 128 : (et + 1) * 128],
                                rhs=key_sb[:, dc, o : o + 512],
                                start=(dc == 0),
                                stop=(dc == 3),
                            )
                    nc.vector.tensor_scalar(
                        kT_sb[:, et, kn * 1024 : (kn + 1) * 1024], ps[:],
                        bk_sb[:, et : et + 1], None, OP.add,
                    )

            # v[s, e] = sum_d v^T[d, s] WvT[d, e]   (bias folded into bo')
            for st in range(16):
                psv = psB.tile([128, 512], f32, tag="psB")
                for dc in range(4):
                    nc.tensor.matmul(
                        psv[:],
                        lhsT=val_sb[:, dc, st * 128 : (st + 1) * 128],
                        rhs=wv_sb[:, dc, :],
                        start=(dc == 0),
                        stop=(dc == 3),
                    )
                nc.vector.tensor_copy(
                    v_sb[:, st, :, 0:HD],
                    psv[:].rearrange("p (h d) -> p h d", h=H),
                )

            # ---- attention (head pairs share one 128-row tile) ------------
            outT = []  # 4 pair tiles [128, SQ] = attn-out^T, normalized
            for hp in range(4):
                pair_out = acts.tile([128, SQ], bf16, tag=f"outT{hp}")
                outT.append(pair_out)
                av = [
                    [
                        psB.tile(
                            [HD + 1, 512], f32, tag="psB",
                            name=f"av{hp}_{hh}_{qc}",
                        )
                        for qc in range(2)
                    ]
                    for hh in range(2)
                ]
                exp_tiles = [[None] * 16, [None] * 16]
                for kt in range(16):
                    st_ps = [None, None]
                    for hh in range(2):
                        lo = 64 * hh
                        st_ps[hh] = psA.tile(
                            [128, SQ], f32, tag="psA", name=f"st{hp}_{kt}_{hh}"
                        )
                        for qn in range(2):
                            nc.tensor.matmul(
                                st_ps[hh][:, qn * 512 : (qn + 1) * 512],
                                lhsT=kT_sb[lo : lo + 64, hp, kt * 128 : (kt + 1) * 128],
                                rhs=qT_sb[lo : lo + 64, hp, qn * 512 : (qn + 1) * 512],
                                start=True,
                                stop=True,
                            )
                    for hh in range(2):
                        e = expp.tile([128, SQ], bf16, tag="exp")
                        exp_tiles[hh][kt] = e
                        nc.scalar.activation(e[:], st_ps[hh][:], AF.Exp)
                    for hh in range(2):
                        h = 2 * hp + hh
                        for qc in range(2):
                            nc.tensor.matmul(
                                av[hh][qc][:],
                                lhsT=v_sb[:, kt, h, :],
                                rhs=exp_tiles[hh][kt][:, qc * 512 : (qc + 1) * 512],
                                start=(kt == 0),
                                stop=(kt == 15),
                            )
                # normalize: out^T[dh, q] = av[dh, q] / av[64, q]
                # Copy PSUM -> SBUF first so the accumulators recycle fast
                # (keeps the PE fed across pair boundaries), then do the
                # recip/broadcast/multiply entirely in SBUF off the critical
                # path.
                for hh in range(2):
                    avsb = rpool.tile([HD + 1, SQ], f32, tag="avsb",
                                      name=f"avsb{hp}_{hh}")
                    for qc in range(2):
                        nc.vector.tensor_copy(
                            avsb[:, qc * 512 : (qc + 1) * 512], av[hh][qc][:]
                        )
                    rrow = rpool.tile([1, SQ], f32, tag="rrow",
                                      name=f"rrow{hp}_{hh}")
                    nc.vector.reciprocal(
                        rrow[:], avsb[HD : HD + 1, :]
                    )
                    scr = dramp.tile([1, SQ], f32, tag="scr",
                                     name=f"scr{hp}_{hh}")
                    nc.sync.dma_start(scr[:], rrow[:])
                    rb = rpool.tile([64, SQ], f32, tag="rb",
                                    name=f"rb{hp}_{hh}")
                    nc.sync.dma_start(rb[:], scr[:].to_broadcast((64, SQ)))
                    nc.vector.tensor_tensor(
                        pair_out[64 * hh : 64 * hh + 64, :],
                        avsb[0:HD, :],
                        rb[:],
                        OP.mult,
                    )

            # ---- output projection ---------------------------------------
            # y[q, o] = sum_e outT[e, q] WoT[e, o] + bo'
            for stq in range(8):
                psy = psA.tile([128, 512], f32, tag="psA")
                for c in range(4):
                    nc.tensor.matmul(
                        psy[:],
                        lhsT=outT[c][:, stq * 128 : (stq + 1) * 128],
                        rhs=wo_sb[:, c, :],
                        start=(c == 0),
                        stop=False,
                    )
                nc.tensor.matmul(
                    psy[:], lhsT=ones_row[:], rhs=bop_sb[:], start=False, stop=True,
                )
                ysb = rpool.tile([128, 512], f32, tag="ysb", name=f"ysb{stq}")
                nc.vector.tensor_copy(ysb[:], psy[:])
                nc.sync.dma_start(y[stq * 128 : (stq + 1) * 128, :], ysb[:])

    nc.compile()
    return nc


def _get_nc():
    if "nc" not in _cache:
        _cache["nc"] = _build()
    return _cache["nc"]


def _host_prep(query, key, value, Wq, bq, Wk, bk, Wv, bv, Wo, bo):
    """Shard + transpose + cast inputs for the 8 cores."""
    bf = ml_dtypes.bfloat16
    wqT = np.ascontiguousarray(Wq.T).astype(bf)
    wkT = np.ascontiguousarray(Wk.T).astype(bf)
    wvT = np.ascontiguousarray(Wv.T).astype(bf)
    woT = np.ascontiguousarray(Wo.T).astype(bf)
    bqr = np.ascontiguousarray(bq.reshape(4, 128).T).astype(np.float32)
    bkr = np.ascontiguousarray(bk.reshape(4, 128).T).astype(np.float32)
    bop = (bo + Wo @ bv).astype(np.float32).reshape(1, D).astype(bf)

    in_maps = []
    for c in range(N_CORES):
        b, half = divmod(c, 2)
        xqT = np.ascontiguousarray(
            query[b, half * SQ : (half + 1) * SQ, :].T
        ).astype(bf)
        keyT = np.ascontiguousarray(key[b].T).astype(bf)
        valT = np.ascontiguousarray(value[b].T).astype(bf)
        in_maps.append(
            {
                "xqT": xqT, "keyT": keyT, "valT": valT,
                "wqT": wqT, "wkT": wkT, "wvT": wvT, "woT": woT,
                "bqr": bqr, "bkr": bkr, "bop": bop,
            }
        )
    return in_maps


def _assemble(results):
    out = np.empty((B, S, D), np.float32)
    for c in range(N_CORES):
        b, half = divmod(c, 2)
        out[b, half * SQ : (half + 1) * SQ, :] = results[c]["y"]
    return out


def _run(in_maps, **spmd_kwargs):
    from concourse.bass_utils import run_bass_kernel_spmd

    nc = _get_nc()
    return run_bass_kernel_spmd(nc, in_maps, list(range(N_CORES)), **spmd_kwargs)


def _reference_fallback(query, key, value, mask, Wq, bq, Wk, bk, Wv, bv, Wo, bo):
    """Exact numpy path, used only if the mask is not all-ones."""
    q = (query @ Wq.T + bq).reshape(B, S, H, HD).transpose(0, 2, 1, 3)
    k = (key @ Wk.T + bk).reshape(B, S, H, HD).transpose(0, 2, 1, 3)
    v = (value @ Wv.T + bv).reshape(B, S, H, HD).transpose(0, 2, 1, 3)
    scores = np.einsum("bhqd,bhkd->bhqk", q, k) / np.sqrt(HD).astype(np.float32)
    scores = np.where(mask[:, None, :, :] == 0, -np.inf, scores)
    scores = scores - scores.max(axis=-1, keepdims=True)
    e = np.exp(scores)
    attn = e / e.sum(axis=-1, keepdims=True)
    x = np.einsum("bhqk,bhkd->bhqd", attn, v)
    x = x.transpose(0, 2, 1, 3).reshape(B, S, D)
    return (x @ Wo.T + bo).astype(np.float32)


def kernel(query, key, value, mask, Wq, bq, Wk, bk, Wv, bv, Wo, bo):
    query = np.asarray(query, np.float32)
    key = np.asarray(key, np.float32)
    value = np.asarray(value, np.float32)
    mask_np = np.asarray(mask)
    args = [
        np.asarray(a, np.float32)
        for a in (Wq, bq, Wk, bk, Wv, bv, Wo, bo)
    ]
    if not np.all(mask_np != 0):
        return _reference_fallback(query, key, value, mask_np, *args)
    in_maps = _host_prep(query, key, value, *args)
    res = _run(in_maps, trace=False)
    return _assemble(res.results)


# revision 9
# speedup vs baseline: 1.1165x; 1.0534x over previous
"""Multi-head attention (B=4, S=2048, D=512, H=8) on 8 Trainium2 NeuronCores.

Sharding: core c handles batch b = c//2 and query-half h = c%2 (1024 queries).
Each core computes q = (x_q @ Wq.T + bq)/sqrt(hd) for its queries, k/v
projections for its batch's full 2048 keys, full softmax attention for all 8
heads, and the output projection for its query rows.  Output rows across
cores are disjoint, so there are no collectives.

On-chip layout is feature-major ("transposed activations"): scores are built
directly as S^T[k, q] so the attn @ V contraction needs no transposes, and
exp(S^T) row-sums come free via a ones-column appended to V.
"""

import numpy as np
import ml_dtypes

B = 4
S = 2048
D = 512
H = 8
HD = 64
SQ = 1024  # queries per core
N_CORES = 8
F32 = None  # set lazily (mybir dtypes) in _build
BF16 = None

_cache = {}


def _build():
    """Build (once) the SPMD Bass program shared by all 8 cores."""
    import concourse.bacc as bacc
    import concourse.mybir as mybir
    import concourse.tile as tile

    f32 = mybir.dt.float32
    bf16 = mybir.dt.bfloat16
    AF = mybir.ActivationFunctionType
    OP = mybir.AluOpType

    nc = bacc.Bacc("TRN2", target_bir_lowering=False, debug=False)

    # Per-core inputs (pre-transposed / pre-cast on host).
    xqT = nc.dram_tensor("xqT", [D, SQ], bf16, kind="ExternalInput").ap()
    keyT = nc.dram_tensor("keyT", [D, S], bf16, kind="ExternalInput").ap()
    valT = nc.dram_tensor("valT", [D, S], bf16, kind="ExternalInput").ap()
    wqT = nc.dram_tensor("wqT", [D, D], bf16, kind="ExternalInput").ap()
    wkT = nc.dram_tensor("wkT", [D, D], bf16, kind="ExternalInput").ap()
    wvT = nc.dram_tensor("wvT", [D, D], bf16, kind="ExternalInput").ap()
    woT = nc.dram_tensor("woT", [D, D], bf16, kind="ExternalInput").ap()
    bqr = nc.dram_tensor("bqr", [128, 4], f32, kind="ExternalInput").ap()
    bkr = nc.dram_tensor("bkr", [128, 4], f32, kind="ExternalInput").ap()
    bop = nc.dram_tensor("bop", [1, D], bf16, kind="ExternalInput").ap()
    y = nc.dram_tensor("y", [SQ, D], f32, kind="ExternalOutput").ap()

    with tile.TileContext(nc) as tc:
        import contextlib

        with contextlib.ExitStack() as ctx:
            const = ctx.enter_context(tc.tile_pool(name="const", bufs=1))
            io = ctx.enter_context(tc.tile_pool(name="io", bufs=1))
            acts = ctx.enter_context(tc.tile_pool(name="acts", bufs=1))
            expp = ctx.enter_context(tc.tile_pool(name="expp", bufs=12))
            rpool = ctx.enter_context(tc.tile_pool(name="rpool", bufs=2))
            dramp = ctx.enter_context(
                tc.tile_pool(name="dramp", bufs=2, space="DRAM")
            )
            psA = ctx.enter_context(tc.tile_pool(name="psA", bufs=2, space="PSUM"))
            psB = ctx.enter_context(tc.tile_pool(name="psB", bufs=4, space="PSUM"))

            # ---- constants / weights -------------------------------------
            wq_sb = const.tile([128, 4, D], bf16)
            wk_sb = const.tile([128, 4, D], bf16)
            wv_sb = const.tile([128, 4, D], bf16)
            wo_sb = const.tile([128, 4, D], bf16)
            for w_sb, w_dr in ((wq_sb, wqT), (wk_sb, wkT), (wv_sb, wvT), (wo_sb, woT)):
                nc.sync.dma_start(w_sb[:], w_dr.rearrange("(c p) e -> p c e", p=128))
            bq_sb = const.tile([128, 4], f32)
            bk_sb = const.tile([128, 4], f32)
            nc.sync.dma_start(bq_sb[:], bqr[:])
            nc.sync.dma_start(bk_sb[:], bkr[:])
            bop_sb = const.tile([1, D], bf16)
            nc.sync.dma_start(bop_sb[:], bop[:])
            ones_row = const.tile([1, 128], bf16)
            nc.vector.memset(ones_row[:], 1.0)

            # ---- inputs ---------------------------------------------------
            xq_sb = io.tile([128, 4, SQ], bf16)
            key_sb = io.tile([128, 4, S], bf16)
            val_sb = io.tile([128, 4, S], bf16)
            nc.sync.dma_start(xq_sb[:], xqT.rearrange("(c p) s -> p c s", p=128))
            nc.sync.dma_start(key_sb[:], keyT.rearrange("(c p) s -> p c s", p=128))
            nc.sync.dma_start(val_sb[:], valT.rearrange("(c p) s -> p c s", p=128))

            # ---- projections ---------------------------------------------
            qT_sb = acts.tile([128, 4, SQ], bf16)  # q^T / 8, feature-major
            kT_sb = acts.tile([128, 4, S], bf16)  # k^T, feature-major
            # v natural [s, e] per k-tile, 65th column = 1.0 (row-sum trick)
            v_sb = acts.tile([128, 16, H, HD + 1], bf16)
            nc.vector.memset(v_sb[:, :, :, HD : HD + 1], 1.0)

            # q^T[e, s] = sum_d WqT[d, e] x^T[d, s]
            def emit_qproj(et):
                ps = psA.tile([128, SQ], f32, tag="psA", name=f"psq{et}")
                for dc in range(4):
                    for qn in range(2):
                        nc.tensor.matmul(
                            ps[:, qn * 512 : (qn + 1) * 512],
                            lhsT=wq_sb[:, dc, et * 128 : (et + 1) * 128],
                            rhs=xq_sb[:, dc, qn * 512 : (qn + 1) * 512],
                            start=(dc == 0),
                            stop=(dc == 3),
                        )
                nc.vector.tensor_scalar(
                    qT_sb[:, et, :], ps[:], bq_sb[:, et : et + 1], 0.125,
                    OP.add, OP.mult,
                )

            # k^T[e, s] likewise (no scale)
            def emit_kproj(et):
                for kn in range(2):
                    ps = psA.tile([128, SQ], f32, tag="psA", name=f"psk{et}_{kn}")
                    for dc in range(4):
                        for qn in range(2):
                            o = kn * 1024 + qn * 512
                            nc.tensor.matmul(
                                ps[:, qn * 512 : (qn + 1) * 512],
                                lhsT=wk_sb[:, dc, et * 128 : (et + 1) * 128],
                                rhs=key_sb[:, dc, o : o + 512],
                                start=(dc == 0),
                                stop=(dc == 3),
                            )
                    nc.vector.tensor_scalar(
                        kT_sb[:, et, kn * 1024 : (kn + 1) * 1024], ps[:],
                        bk_sb[:, et : et + 1], None, OP.add,
                    )

            # v[s, e] = sum_d v^T[d, s] WvT[d, e]   (bias folded into bo')
            def emit_vproj(st):
                psv = psA.tile([128, 512], f32, tag="psA", name=f"psv{st}")
                for dc in range(4):
                    nc.tensor.matmul(
                        psv[:],
                        lhsT=val_sb[:, dc, st * 128 : (st + 1) * 128],
                        rhs=wv_sb[:, dc, :],
                        start=(dc == 0),
                        stop=(dc == 3),
                    )
                nc.vector.tensor_copy(
                    v_sb[:, st, :, 0:HD],
                    psv[:].rearrange("p (h d) -> p h d", h=H),
                )

            # pair 0 only needs q/k features et=0: emit those first so the
            # Scalar engine (softmax exp, the co-bottleneck) starts early.
            for _et in range(4):
                emit_qproj(_et)
                emit_kproj(_et)
            for _st in range(16):
                emit_vproj(_st)

            # ---- attention (head pairs share one 128-row tile) ------------
            outT = []  # 4 pair tiles [128, SQ] = attn-out^T, normalized
            for hp in range(4):
                pair_out = acts.tile([128, SQ], bf16, tag=f"outT{hp}")
                outT.append(pair_out)
                av = [
                    [
                        psB.tile(
                            [HD + 1, 512], f32, tag="psB",
                            name=f"av{hp}_{hh}_{qc}",
                        )
                        for qc in range(2)
                    ]
                    for hh in range(2)
                ]
                exp_tiles = [[None] * 16, [None] * 16]
                for kt in range(16):
                    st_ps = [None, None]
                    for hh in range(2):
                        lo = 64 * hh
                        st_ps[hh] = psA.tile(
                            [128, SQ], f32, tag="psA", name=f"st{hp}_{kt}_{hh}"
                        )
                        for qn in range(2):
                            nc.tensor.matmul(
                                st_ps[hh][:, qn * 512 : (qn + 1) * 512],
                                lhsT=kT_sb[lo : lo + 64, hp, kt * 128 : (kt + 1) * 128],
                                rhs=qT_sb[lo : lo + 64, hp, qn * 512 : (qn + 1) * 512],
                                start=True,
                                stop=True,
                            )
                    for hh in range(2):
                        e = expp.tile([128, SQ], bf16, tag="exp")
                        exp_tiles[hh][kt] = e
                        nc.scalar.activation(e[:], st_ps[hh][:], AF.Exp)
                    for hh in range(2):
                        h = 2 * hp + hh
                        for qc in range(2):
                            nc.tensor.matmul(
                                av[hh][qc][:],
                                lhsT=v_sb[:, kt, h, :],
                                rhs=exp_tiles[hh][kt][:, qc * 512 : (qc + 1) * 512],
                                start=(kt == 0),
                                stop=(kt == 15),
                            )
                # normalize: out^T[dh, q] = av[dh, q] / av[64, q]
                # Copy PSUM -> SBUF first so the accumulators recycle fast
                # (keeps the PE fed across pair boundaries), then do the
                # recip/broadcast/multiply entirely in SBUF off the critical
                # path.
                avsbs = []
                for hh in range(2):
                    avsb = rpool.tile([HD + 1, SQ], f32, tag="avsb",
                                      name=f"avsb{hp}_{hh}")
                    avsbs.append(avsb)
                    for qc in range(2):
                        nc.vector.tensor_copy(
                            avsb[:, qc * 512 : (qc + 1) * 512], av[hh][qc][:]
                        )
                for hh in range(2):
                    avsb = avsbs[hh]
                    rrow = rpool.tile([1, SQ], f32, tag="rrow",
                                      name=f"rrow{hp}_{hh}")
                    nc.vector.reciprocal(
                        rrow[:], avsb[HD : HD + 1, :]
                    )
                    scr = dramp.tile([1, SQ], f32, tag="scr",
                                     name=f"scr{hp}_{hh}")
                    nc.sync.dma_start(scr[:], rrow[:])
                    rb = rpool.tile([64, SQ], f32, tag="rb",
                                    name=f"rb{hp}_{hh}")
                    nc.sync.dma_start(rb[:], scr[:].to_broadcast((64, SQ)))
                    nc.vector.tensor_tensor(
                        pair_out[64 * hh : 64 * hh + 64, :],
                        avsb[0:HD, :],
                        rb[:],
                        OP.mult,
                    )

            # ---- output projection ---------------------------------------
            # y[q, o] = sum_e outT[e, q] WoT[e, o] + bo'
            for stq in range(8):
                psy = psA.tile([128, 512], f32, tag="psA")
                for c in range(4):
                    nc.tensor.matmul(
                        psy[:],
                        lhsT=outT[c][:, stq * 128 : (stq + 1) * 128],
                        rhs=wo_sb[:, c, :],
                        start=(c == 0),
                        stop=False,
                    )
                nc.tensor.matmul(
                    psy[:], lhsT=ones_row[:], rhs=bop_sb[:], start=False, stop=True,
                )
                ysb = rpool.tile([128, 512], f32, tag="ysb", name=f"ysb{stq}")
                nc.vector.tensor_copy(ysb[:], psy[:])
                nc.sync.dma_start(y[stq * 128 : (stq + 1) * 128, :], ysb[:])

    nc.compile()
    return nc


def _get_nc():
    if "nc" not in _cache:
        _cache["nc"] = _build()
    return _cache["nc"]


def _host_prep(query, key, value, Wq, bq, Wk, bk, Wv, bv, Wo, bo):
    """Shard + transpose + cast inputs for the 8 cores."""
    bf = ml_dtypes.bfloat16
    wqT = np.ascontiguousarray(Wq.T).astype(bf)
    wkT = np.ascontiguousarray(Wk.T).astype(bf)
    wvT = np.ascontiguousarray(Wv.T).astype(bf)
    woT = np.ascontiguousarray(Wo.T).astype(bf)
    bqr = np.ascontiguousarray(bq.reshape(4, 128).T).astype(np.float32)
    bkr = np.ascontiguousarray(bk.reshape(4, 128).T).astype(np.float32)
    bop = (bo + Wo @ bv).astype(np.float32).reshape(1, D).astype(bf)

    in_maps = []
    for c in range(N_CORES):
        b, half = divmod(c, 2)
        xqT = np.ascontiguousarray(
            query[b, half * SQ : (half + 1) * SQ, :].T
        ).astype(bf)
        keyT = np.ascontiguousarray(key[b].T).astype(bf)
        valT = np.ascontiguousarray(value[b].T).astype(bf)
        in_maps.append(
            {
                "xqT": xqT, "keyT": keyT, "valT": valT,
                "wqT": wqT, "wkT": wkT, "wvT": wvT, "woT": woT,
                "bqr": bqr, "bkr": bkr, "bop": bop,
            }
        )
    return in_maps


def _assemble(results):
    out = np.empty((B, S, D), np.float32)
    for c in range(N_CORES):
        b, half = divmod(c, 2)
        out[b, half * SQ : (half + 1) * SQ, :] = results[c]["y"]
    return out


def _run(in_maps, **spmd_kwargs):
    from concourse.bass_utils import run_bass_kernel_spmd

    nc = _get_nc()
    return run_bass_kernel_spmd(nc, in_maps, list(range(N_CORES)), **spmd_kwargs)


def _reference_fallback(query, key, value, mask, Wq, bq, Wk, bk, Wv, bv, Wo, bo):
    """Exact numpy path, used only if the mask is not all-ones."""
    q = (query @ Wq.T + bq).reshape(B, S, H, HD).transpose(0, 2, 1, 3)
    k = (key @ Wk.T + bk).reshape(B, S, H, HD).transpose(0, 2, 1, 3)
    v = (value @ Wv.T + bv).reshape(B, S, H, HD).transpose(0, 2, 1, 3)
    scores = np.einsum("bhqd,bhkd->bhqk", q, k) / np.sqrt(HD).astype(np.float32)
    scores = np.where(mask[:, None, :, :] == 0, -np.inf, scores)
    scores = scores - scores.max(axis=-1, keepdims=True)
    e = np.exp(scores)
    attn = e / e.sum(axis=-1, keepdims=True)
    x = np.einsum("bhqk,bhkd->bhqd", attn, v)
    x = x.transpose(0, 2, 1, 3).reshape(B, S, D)
    return (x @ Wo.T + bo).astype(np.float32)


def kernel(query, key, value, mask, Wq, bq, Wk, bk, Wv, bv, Wo, bo):
    query = np.asarray(query, np.float32)
    key = np.asarray(key, np.float32)
    value = np.asarray(value, np.float32)
    mask_np = np.asarray(mask)
    args = [
        np.asarray(a, np.float32)
        for a in (Wq, bq, Wk, bk, Wv, bv, Wo, bo)
    ]
    if not np.all(mask_np != 0):
        return _reference_fallback(query, key, value, mask_np, *args)
    in_maps = _host_prep(query, key, value, *args)
    res = _run(in_maps, trace=False)
    return _assemble(res.results)
